# revision 2
# baseline (speedup 1.0000x reference)
"""Self-contained TRN2 kernel for the bidirectional attention correction.

kernel(hl, hr) -> (mu_lr, mu_rl), matching:
    hl_n = rownorm(hl); hr_n = rownorm(hr)
    a = hl_n @ hr_n.T
    mu_lr = hr_n - softmax(a, 1).T @ hl_n
    mu_rl = hl_n - softmax(a, 0) @ hr_n

Runs SPMD on 8 NeuronCores: core c owns rows [c*1024,(c+1)*1024) of hl and
hr. All three 8192x8192x1024 GEMMs run in fp8 DoubleRow. exp(a) is computed
in both orientations ([n,m] for the row-softmax GEMM, [m,n] for the
column-softmax GEMM) straight from the gathered fp8 operands, so no PE
transposes of the 8 MB exp array are needed; softmax sums fall out of the
activation accumulators.
"""

import sys

for _p in ("/opt/trn_rl_repo",):
    if _p not in sys.path:
        sys.path.insert(0, _p)

from contextlib import ExitStack

import numpy as np

import concourse.bass as bass
import concourse.tile as tile
from concourse import bacc, mybir
from concourse.masks import make_identity

F32 = mybir.dt.float32
BF16 = mybir.dt.bfloat16
FP8 = mybir.dt.float8e4

ADD = mybir.AluOpType.add
MULT = mybir.AluOpType.mult
BYPASS = mybir.AluOpType.bypass
EXP = mybir.ActivationFunctionType.Exp
COPY = mybir.ActivationFunctionType.Copy
SQUARE = mybir.ActivationFunctionType.Square
AXL_X = mybir.AxisListType.X
DROW = mybir.MatmulPerfMode.DoubleRow


def build(C=8, NL=1024, M=8192, D=1024, stop_after="full"):
    """Build + compile the SPMD Bass graph."""
    PB = NL // 128          # local row blocks (8)
    DK = D // 128           # 128-chunks over D (8)
    JB = M // 128           # j 128-blocks (64)
    W1 = 512                # j-chunk width
    NLH = NL // 2           # j-cols per gather half (512)
    DW = 512                # d-half width
    DH = D // DW            # 2
    SP = 64.0               # fp8 scale on P1 operands
    SE = SP * SP            # a-scale in psum (4096)
    S1 = float(8 * M)       # hl' fp8 scale
    S2 = float(M // 2)      # exp_aT sinv scale
    S8 = 8.0                # hrn8 fp8 scale
    groups = [list(range(C))]
    LVL = {"prep": 0, "p1": 1, "p2a": 2, "full": 3}[stop_after]

    nc = bacc.Bacc("TRN2", target_bir_lowering=False, debug=False, num_devices=C)

    hl_in = nc.dram_tensor("hl", [NL, D], F32, kind="ExternalInput").ap()
    hr_in = nc.dram_tensor("hr", [NL, D], F32, kind="ExternalInput").ap()
    mu_lr_o = nc.dram_tensor("mu_lr", [NL, D], F32, kind="ExternalOutput").ap()
    mu_rl_o = nc.dram_tensor("mu_rl", [NL, D], F32, kind="ExternalOutput").ap()

    with tile.TileContext(nc) as tc, ExitStack() as ctx:
        dram = ctx.enter_context(tc.tile_pool(name="dram", bufs=1, space="DRAM"))
        sb = ctx.enter_context(tc.tile_pool(name="sb", bufs=1))
        ps = ctx.enter_context(tc.tile_pool(name="ps", bufs=1, space="PSUM"))

        # ---- internal DRAM ----
        hln_d = dram.tile([NL, D], BF16)           # hl_n rows (restreamed)
        hrn_d = dram.tile([NL, D], BF16)           # hr_n rows (restreamed)
        hrnT_loc = [dram.tile([D, NLH], FP8, name=f"hrnT_loc{h}")
                    for h in range(2)]
        hrnT_all = [dram.tile([C, D, NLH], FP8, name=f"hrnT_all{h}",
                              addr_space="Shared") for h in range(2)]
        hrn8_loc = dram.tile([NL, D], FP8)
        hrn8_all = dram.tile([C, NL, D], FP8, addr_space="Shared")
        s_loc = dram.tile([M], F32)
        s_glob = dram.tile([M], F32, addr_space="Shared")
        vlr_h = [dram.tile([M // 2, D], BF16, name=f"vlr_h{x}")
                 for x in range(2)]
        vred_h = [dram.tile([NL // 2, D], BF16, name=f"vred_h{x}")
                  for x in range(2)]

        # ---- SBUF resident ----
        exp_a = sb.tile([128, PB, M], FP8, name="exp_a")       # exp(a) [n,m]
        exp_aT = sb.tile([128, JB, NL], FP8, name="exp_aT")    # exp(a.T) [m,n]
        hl_nT = sb.tile([128, DK, NL], FP8, name="hl_nT")      # (hl_n*SP).T
        hlp8 = sb.tile([128, PB, D], FP8, name="hlp8")         # hl_n*rinv*S1
        # streaming / staging (manual slot rotation)
        rt = sb.tile([128, 2, DK, W1], FP8, name="rt")         # P1 rhs stream
        rb = sb.tile([128, 2, PB, DW], FP8, name="rb")         # P2b rhs stream
        ld_st = sb.tile([128, 2, D], F32, name="ld_st")
        nrm_st = sb.tile([128, 3, D], BF16, name="nrm_st")
        trT_st = sb.tile([128, 2, DK, 128], FP8, name="trT_st")
        h8_st = sb.tile([128, 2, D], FP8, name="h8_st")
        hlp_st = sb.tile([128, 2, D], BF16, name="hlp_st")
        vlr_st = sb.tile([128, 2, 2, D], BF16, name="vlr_st")
        out_st = sb.tile([128, 2, DW], F32, name="out_st")
        hlb_st = sb.tile([128, 2, DW], BF16, name="hlb_st")
        fin_st = sb.tile([128, 2, DW], F32, name="fin_st")
        vred_st = sb.tile([128, 2, DW], BF16, name="vred_st")
        hrn_st = sb.tile([128, 2, DW], BF16, name="hrn_st")
        # consts / stats
        ident_b = sb.tile([128, 128], BF16, name="ident_b")
        stats = sb.tile([128, 480], F32, name="stats")
        r_parts = stats[:, 0:128].rearrange("p (a b) -> p a b", a=PB)
        r_red = stats[:, 128:128 + PB]
        r_red3 = stats[:, 128:128 + PB].rearrange("p (a b) -> p a b", b=1)
        rinv = stats[:, 136:136 + PB]
        sA = stats[:, 144:208]
        sB = stats[:, 208:272]
        s_sb = stats[:, 272:336]
        srec = stats[:, 336:400]
        sinv = stats[:, 400:464]
        nrm = stats[:, 464:480].rearrange("p (a b) -> p a b", a=8)  # [128,8,2]

        make_identity(nc, ident_b)

        # ================= prep: normalize, transpose, gather ===============
        def norm_chunk(src, pb, it):
            """rownorm one 128-row chunk -> bf16 staging tile."""
            ld = ld_st[:, it % 2, :]
            nc.sync.dma_start(out=ld, in_=src[pb * 128:(pb + 1) * 128, :])
            nm = nrm[:, it % 8, :]
            for dd in range(DH):
                sq = ps.tile([128, DW], F32, tag="rot", bufs=4,
                             name=f"sq{it}_{dd}")
                nc.scalar.activation(out=sq, in_=ld[:, dd * DW:(dd + 1) * DW],
                                     func=SQUARE, accum_out=nm[:, dd:dd + 1])
            nc.vector.tensor_add(out=nm[:, 0:1], in0=nm[:, 0:1],
                                 in1=nm[:, 1:2])
            nc.scalar.sqrt(out=nm[:, 1:2], in_=nm[:, 0:1])
            nc.vector.reciprocal(out=nm[:, 0:1], in_=nm[:, 1:2])
            nst = nrm_st[:, it % 3, :]
            nc.vector.tensor_scalar_mul(out=nst, in0=ld, scalar1=nm[:, 0:1])
            return nst

        hrn8_rows = hrn8_loc.rearrange("(pb p) d -> p pb d", p=128)

        def hr_chunk(pb, it):
            nst = norm_chunk(hr_in, pb, it)
            nc.scalar.dma_start(out=hrn_d[pb * 128:(pb + 1) * 128, :], in_=nst)
            t8 = h8_st[:, pb % 2, :]
            nc.gpsimd.tensor_scalar_mul(out=t8, in0=nst, scalar1=S8)
            nc.gpsimd.dma_start(out=hrn8_rows[:, pb, :], in_=t8)
            h, pq = divmod(pb, PB // 2)
            ts = trT_st[:, pb % 2, :, :]
            for dk in range(DK):
                pst = ps.tile([128, 128], BF16, tag="rot", bufs=4,
                              name=f"ptB{pb}_{dk}")
                nc.tensor.transpose(pst, nst[:, dk * 128:(dk + 1) * 128],
                                    ident_b)
                nc.vector.tensor_scalar_mul(out=ts[:, dk, :], in0=pst,
                                            scalar1=SP)
            nc.sync.dma_start(
                out=hrnT_loc[h].rearrange("(dk p) j -> p dk j", p=128)
                [:, :, pq * 128:(pq + 1) * 128],
                in_=ts)

        def hl_chunk(pb, it):
            nst = norm_chunk(hl_in, pb, it)
            nc.scalar.dma_start(out=hln_d[pb * 128:(pb + 1) * 128, :], in_=nst)
            for dk in range(DK):
                pst = ps.tile([128, 128], BF16, tag="rot", bufs=4,
                              name=f"ptA{pb}_{dk}")
                nc.tensor.transpose(pst, nst[:, dk * 128:(dk + 1) * 128],
                                    ident_b)
                nc.vector.tensor_scalar_mul(
                    out=hl_nT[:, dk, pb * 128:(pb + 1) * 128], in0=pst,
                    scalar1=SP)

        for pb in range(PB // 2):
            hr_chunk(pb, pb)
        nc.gpsimd.collective_compute(
            "AllGather", BYPASS, replica_groups=groups,
            ins=[hrnT_loc[0].opt()], outs=[hrnT_all[0].opt()])
        for pb in range(PB // 2, PB):
            hr_chunk(pb, pb)
        nc.gpsimd.collective_compute(
            "AllGather", BYPASS, replica_groups=groups,
            ins=[hrnT_loc[1].opt()], outs=[hrnT_all[1].opt()])
        nc.gpsimd.collective_compute(
            "AllGather", BYPASS, replica_groups=groups,
            ins=[hrn8_loc.opt()], outs=[hrn8_all.opt()])
        for pb in range(PB):
            hl_chunk(pb, PB + pb)

        # ====== P1/P1T: exp(a) [n,m] + exp(a.T) [m,n] + softmax sums ========
        chunks = [(h, b) for h in range(2) for b in range(C)]
        if LVL >= 1:
            for ci, (h, b) in enumerate(chunks):
                j0 = b * NL + h * NLH
                rt_t = rt[:, ci % 2, :, :]
                nc.sync.dma_start(
                    out=rt_t,
                    in_=hrnT_all[h][b].rearrange("(dk p) j -> p dk j", p=128))
                # P1: a rows for all 8 i-blocks of this j-chunk
                for ib in range(PB):
                    pa = ps.tile([128, W1], F32, tag="rot", bufs=4,
                                 name=f"pa{ci}_{ib}")
                    for kp in range(DK // 2):
                        nc.tensor.matmul(
                            pa,
                            lhsT=hl_nT[:, 2 * kp:2 * kp + 2,
                                       ib * 128:(ib + 1) * 128],
                            rhs=rt_t[:, 2 * kp:2 * kp + 2, :],
                            start=(kp == 0), stop=(kp == DK // 2 - 1),
                            perf_mode=DROW)
                    nc.scalar.activation(
                        out=exp_a[:, ib, j0:j0 + W1], in_=pa, func=EXP,
                        scale=1.0 / SE, accum_out=r_parts[:, ib, ci:ci + 1])
                # P1T: a.T rows for the 4 m-blocks of this j-chunk
                for ms in range(W1 // 128):
                    jb = j0 // 128 + ms
                    pts = [ps.tile([128, W1], F32, tag="acc", bufs=4,
                                   name=f"pt{ci}_{ms}_{k}") for k in range(2)]
                    for kp in range(DK // 2):
                        for k in range(2):
                            nc.tensor.matmul(
                                pts[k],
                                lhsT=rt_t[:, 2 * kp:2 * kp + 2,
                                          ms * 128:(ms + 1) * 128],
                                rhs=hl_nT[:, 2 * kp:2 * kp + 2,
                                          k * W1:(k + 1) * W1],
                                start=(kp == 0), stop=(kp == DK // 2 - 1),
                                perf_mode=DROW)
                    for k, sdst in enumerate((sA, sB)):
                        nc.scalar.activation(
                            out=exp_aT[:, jb, k * W1:(k + 1) * W1],
                            in_=pts[k], func=EXP, scale=1.0 / SE,
                            accum_out=sdst[:, jb:jb + 1])

            # r -> rinv ; hl' fp8 (restream hl_n rows)
            nc.vector.tensor_reduce(out=r_red3, in_=r_parts, op=ADD,
                                    axis=AXL_X)
            nc.vector.reciprocal(out=rinv, in_=r_red)
            for pb in range(PB):
                hs = hlp_st[:, pb % 2, :]
                nc.sync.dma_start(out=hs,
                                  in_=hln_d[pb * 128:(pb + 1) * 128, :])
                nc.vector.tensor_scalar(
                    out=hlp8[:, pb, :], in0=hs, scalar1=rinv[:, pb:pb + 1],
                    scalar2=S1, op0=MULT, op1=MULT)

            # s: local sums -> AllReduce -> sinv = S2/s; fold into exp_aT
            nc.vector.tensor_add(out=sA, in0=sA, in1=sB)
            nc.sync.dma_start(
                out=s_loc.rearrange("(b p) -> p b", p=128), in_=sA)
            nc.gpsimd.collective_compute(
                "AllReduce", ADD, replica_groups=groups,
                ins=[s_loc.opt()], outs=[s_glob.opt()])
            nc.sync.dma_start(
                out=s_sb, in_=s_glob.rearrange("(b p) -> p b", p=128))
            nc.vector.reciprocal(out=srec, in_=s_sb)
            nc.vector.tensor_scalar_mul(out=sinv, in0=srec, scalar1=S2)
            for jb in range(JB):
                nc.vector.tensor_scalar_mul(
                    out=exp_aT[:, jb, :], in0=exp_aT[:, jb, :],
                    scalar1=sinv[:, jb:jb + 1])

        # ====== P2a: vlr = exp_a.T @ hl'  (row-permuted halves, 2x RS) ======
        def vlr_row(jb):
            c0, lb = divmod(jb, PB)
            return (0, c0 * 512 + lb * 128) if lb < 4 else \
                   (1, c0 * 512 + (lb - 4) * 128)

        if LVL >= 2:
            a_jbs = [jb for jb in range(JB) if jb % PB < 4]
            b_jbs = [jb for jb in range(JB) if jb % PB >= 4]
            for half, jbs in enumerate((a_jbs, b_jbs)):
                for pi in range(len(jbs) // 2):
                    pair = jbs[2 * pi:2 * pi + 2]
                    vst = vlr_st[:, pi % 2, :, :]
                    for jj, jb in enumerate(pair):
                        pls = [ps.tile([128, DW], F32, tag="acc", bufs=4,
                                       name=f"pl{jb}_{dh}")
                               for dh in range(DH)]
                        for icp in range(PB // 2):
                            for dh in range(DH):
                                nc.tensor.matmul(
                                    pls[dh],
                                    lhsT=exp_a[:, 2 * icp:2 * icp + 2,
                                               jb * 128:(jb + 1) * 128],
                                    rhs=hlp8[:, 2 * icp:2 * icp + 2,
                                             dh * DW:(dh + 1) * DW],
                                    start=(icp == 0),
                                    stop=(icp == PB // 2 - 1),
                                    perf_mode=DROW)
                        for dh in range(DH):
                            nc.scalar.activation(
                                out=vst[:, jj, dh * DW:(dh + 1) * DW],
                                in_=pls[dh], func=COPY, scale=1.0 / S1)
                    _, r0 = vlr_row(pair[0])
                    eng = nc.scalar if pi % 2 == 0 else nc.sync
                    eng.dma_start(
                        out=vlr_h[half][r0:r0 + 256, :].rearrange(
                            "(jj p) d -> p jj d", p=128),
                        in_=vst)
                nc.gpsimd.collective_compute(
                    "ReduceScatter", ADD, replica_groups=groups,
                    ins=[vlr_h[half].opt()], outs=[vred_h[half].opt()])

        # ====== P2b: mu_rl = hl_n - exp_aT_scaled.T @ (hrn8*sinv-folded) ====
        if LVL >= 3:
            for dh in range(DH):
                for ih in range(2):
                    accs = [ps.tile([128, DW], F32, tag="acc", bufs=4,
                                    name=f"acc{dh}_{ih}_{xi}")
                            for xi in range(4)]
                    for bb in range(C):
                        rbt = rb[:, bb % 2, :, :]
                        eng = nc.sync if bb % 2 == 0 else nc.scalar
                        eng.dma_start(
                            out=rbt,
                            in_=hrn8_all[bb].rearrange(
                                "(jb p) d -> p jb d", p=128)
                            [:, :, dh * DW:(dh + 1) * DW])
                        for l_ in range(PB // 2):
                            jbp = bb * 4 + l_
                            for xi in range(4):
                                ib = 4 * ih + xi
                                nc.tensor.matmul(
                                    accs[xi],
                                    lhsT=exp_aT[:, 2 * jbp:2 * jbp + 2,
                                                ib * 128:(ib + 1) * 128],
                                    rhs=rbt[:, 2 * l_:2 * l_ + 2, :],
                                    start=(jbp == 0), stop=(jbp == JB // 2 - 1),
                                    perf_mode=DROW)
                    for xi in range(4):
                        ib = 4 * ih + xi
                        st = out_st[:, xi % 2, :]
                        hb = hlb_st[:, xi % 2, :]
                        nc.sync.dma_start(
                            out=hb, in_=hln_d[ib * 128:(ib + 1) * 128,
                                              dh * DW:(dh + 1) * DW])
                        nc.scalar.activation(out=st, in_=accs[xi], func=COPY,
                                             scale=-1.0 / (S2 * S8))
                        nc.vector.tensor_add(out=st, in0=st, in1=hb)
                        nc.scalar.dma_start(
                            out=mu_rl_o[ib * 128:(ib + 1) * 128,
                                        dh * DW:(dh + 1) * DW], in_=st)

        # ================= final: mu_lr = hr_n - vred =======================
        if LVL >= 2:
            for pb in range(PB):
                half, r0 = (0, pb * 128) if pb < 4 else (1, (pb - 4) * 128)
                for dh in range(DH):
                    it = pb * DH + dh
                    vs = vred_st[:, it % 2, :]
                    nc.sync.dma_start(
                        out=vs, in_=vred_h[half][r0:r0 + 128,
                                                 dh * DW:(dh + 1) * DW])
                    hs = hrn_st[:, it % 2, :]
                    nc.sync.dma_start(
                        out=hs, in_=hrn_d[pb * 128:(pb + 1) * 128,
                                          dh * DW:(dh + 1) * DW])
                    st = fin_st[:, it % 2, :]
                    nc.vector.tensor_sub(out=st, in0=hs, in1=vs)
                    nc.sync.dma_start(
                        out=mu_lr_o[pb * 128:(pb + 1) * 128,
                                    dh * DW:(dh + 1) * DW], in_=st)

        # dummy writes for any output a stopped-early build didn't produce
        if LVL < 3:
            for pb in range(PB):
                for dh in range(DH):
                    st = out_st[:, pb % 2, :]
                    nc.vector.memset(st, 0.0)
                    nc.sync.dma_start(
                        out=mu_rl_o[pb * 128:(pb + 1) * 128,
                                    dh * DW:(dh + 1) * DW], in_=st)
        if LVL < 2:
            for pb in range(PB):
                for dh in range(DH):
                    st = fin_st[:, pb % 2, :]
                    nc.vector.memset(st, 0.0)
                    nc.sync.dma_start(
                        out=mu_lr_o[pb * 128:(pb + 1) * 128,
                                    dh * DW:(dh + 1) * DW], in_=st)

    nc.compile()
    return nc


_NC_CACHE = {}


def _get_nc():
    if "nc" not in _NC_CACHE:
        _NC_CACHE["nc"] = build(C=8, NL=1024, M=8192, D=1024)
    return _NC_CACHE["nc"]


def kernel(hl, hr):
    """Full inputs in, full outputs out; distributes across 8 cores."""
    from concourse.bass_utils import run_bass_kernel_spmd

    C, NL = 8, 1024
    hl = np.ascontiguousarray(np.asarray(hl, dtype=np.float32))
    hr = np.ascontiguousarray(np.asarray(hr, dtype=np.float32))
    nc = _get_nc()
    in_maps = [
        {"hl": np.ascontiguousarray(hl[c * NL:(c + 1) * NL]),
         "hr": np.ascontiguousarray(hr[c * NL:(c + 1) * NL])}
        for c in range(C)
    ]
    res = run_bass_kernel_spmd(nc, in_maps, list(range(C)))
    mu_lr = np.concatenate([res.results[c]["mu_lr"] for c in range(C)])
    mu_rl = np.concatenate([res.results[c]["mu_rl"] for c in range(C)])
    return mu_lr, mu_rl
